# revision 3
# baseline (speedup 1.0000x reference)
"""MoE down-proj GEMM + reduce over tensor-parallel K-shards (the R axis),
as a Bass/Tile kernel SPMD across 8 TRN2 NeuronCores.

Problem (hardcoded from the spec):
    intermediate_states [4, 4096, 1024] f16, w [4, 8, 1024, 4096] f16,
    topk_ids [2048, 2] i32, topk_weight [2048, 2] f32 -> out [2048, 4096] f16

Strategy (no collectives needed):
- Tensor-parallel on the OUTPUT dim: every core owns a 512-wide slice of
  H; the reduce over the R axis (the module's tensor-parallel K-shards)
  happens inside each core's PSUM accumulation (4 ranks x 8 k-tiles = 32
  accumulating matmuls per 128-token m-tile).
- Host routing: the GEMM is linear in x, so a token whose TOPK=2 slots
  hit the SAME expert is pre-combined into one row tw1*x1+tw2*x2 (weight
  1.0). Entries are stably sorted by expert and each expert group padded
  to 128-row m-tiles, so each matmul's stationary operand is
  single-expert. Activations are pre-transposed to [k, m] lhsT layout.
- Eviction applies the router weight as a per-partition ACT scale (fp16)
  and indirect-scatter-ADDs the rows straight into the pre-zeroed output
  (DMA CCE add): the TOPK pair-sum happens inside the DMA. Within one
  scatter all target rows are distinct (after combining, a token has at
  most one slot per expert), and Tile serializes the scatter chain, so
  there are no read-modify-write races. Padded rows carry x=0 and
  therefore add exactly 0.0 wherever they land (row 0).
- No second pass; the host just concatenates the 8 H-slices.
"""
import numpy as np

import concourse.bass as bass
import concourse.mybir as mybir
from concourse import tile
from concourse.bass import IndirectOffsetOnAxis

R, T, TOPK, KP, E, H = 4, 2048, 2, 1024, 8, 4096
MT = T * TOPK
NC = 8
HC = H // NC             # 512 H-columns per core
KT = KP // 128           # 8 k-tiles
P = 128

FP16 = mybir.dt.float16
FP32 = mybir.dt.float32
INT32 = mybir.dt.int32


# ------------------------------------------------------------- walrus workarounds
def install_drain_patch():
    """TileContext's exit drain waits on the whole 27-proc global clock in
    one instruction; this walrus build rejects >1 sync-wait per instruction
    (and any ge-imm wait on a Drain). Split into single-wait SP nops."""
    import bass_rust

    def _split_drain_and_barrier(self, tick_clock, wait_clock):
        ScopedClock = bass_rust.ScopedClock
        g = list(tick_clock.global_clock)
        for i, v in enumerate(g):
            if v <= 0:
                continue
            vc = bass_rust.VectorClock([v if j == i else 0 for j in range(len(g))])
            nop = self.nc.sync.nop(nofuse=True, hint="drain_wait_split")
            wait_clock.add_sem_waits(nop.ins, ScopedClock({None: vc}))
        self.nc.sync.drain()
        self.nc.all_engine_barrier()
        assert self.sems is not None
        popped = self.nc._tile_sem_poison_stack.pop()
        assert popped is self._sem_poison
        self.nc.clear_and_free_semaphores(list(self.sems.allocated().values()))
        self.nc.all_engine_barrier()

    tile.TileContext._drain_and_barrier = _split_drain_and_barrier


def split_multiwait(nc):
    """Hoist extra sync-waits onto fresh same-engine nops inserted just
    before the instruction (walrus encodes at most one wait each)."""
    for bb in nc.main_func.blocks:
        insts = list(bb.instructions)
        if not any(
            ins.sync_info and ins.sync_info.on_wait and len(ins.sync_info.on_wait) > 1
            for ins in insts
        ):
            continue
        out = []
        for ins in insts:
            si = ins.sync_info
            waits = list(si.on_wait) if si and si.on_wait else []
            if len(waits) > 1:
                eng = nc.engines[ins.engine]
                for wx in waits[:-1]:
                    nop = eng.nop(nofuse=True, hint="wait_split").ins
                    cur = nc.cur_bb.bb
                    lst = cur.instructions
                    assert lst and lst[-1].name == nop.name
                    cur.instructions = lst[:-1]
                    nsi = nop.sync_info
                    if nsi is None:
                        nop.sync_info = mybir.SyncInfo(on_wait=[wx], on_update=[])
                    else:
                        nsi.on_wait = [wx]
                    out.append(nop)
                si.on_wait = waits[-1:]
            out.append(ins)
        bb.instructions = out


# ---------------------------------------------------------------- host routing
def prep_inputs(intermediate_states, w, topk_ids, topk_weight):
    x = np.asarray(intermediate_states)                       # [R, MT, KP] f16
    w = np.asarray(w)                                         # [R, E, KP, H] f16
    tw = np.asarray(topk_weight, np.float32).reshape(T, TOPK)
    ids = np.asarray(topk_ids, np.int64).reshape(T, TOPK)

    comb = ids[:, 0] == ids[:, 1]
    tcomb = np.nonzero(comb)[0]
    tsep = np.nonzero(~comb)[0]

    # entries: combined tokens (one row, weights folded into x), then the
    # two separate slots of mixed-expert tokens
    ent_e = np.concatenate([ids[tcomb, 0], ids[tsep, 0], ids[tsep, 1]])
    ent_tok = np.concatenate([tcomb, tsep, tsep])             # scatter target row
    n_ent = len(ent_e)

    xe = np.empty((R, n_ent, KP), np.float16)
    nco = len(tcomb)
    if nco:
        xe[:, :nco, :] = (
            x[:, 2 * tcomb, :].astype(np.float32) * tw[tcomb, 0][None, :, None]
            + x[:, 2 * tcomb + 1, :].astype(np.float32) * tw[tcomb, 1][None, :, None]
        ).astype(np.float16)
    xe[:, nco:nco + len(tsep), :] = x[:, 2 * tsep, :]
    xe[:, nco + len(tsep):, :] = x[:, 2 * tsep + 1, :]
    ent_tw = np.concatenate([np.ones(nco, np.float32), tw[tsep, 0], tw[tsep, 1]])

    # stable sort by expert; pad each expert group to 128-row m-tiles
    order = np.argsort(ent_e, kind="stable")
    ept = []                                                  # m-tile -> expert
    rows = []                                                 # padded row -> entry (-1 pad)
    for e in range(E):
        grp = order[ent_e[order] == e]
        n = len(grp)
        if n == 0:
            continue
        ntile = -(-n // P)
        padded = np.full(ntile * P, -1, np.int64)
        padded[:n] = grp
        rows.append(padded)
        ept.extend([e] * ntile)
    rows = np.concatenate(rows)
    NT = len(ept)
    NCH = -(-NT // 4)                                         # 512-row chunks
    P4 = NCH * 4 * P
    valid = np.nonzero(rows >= 0)[0]

    # xt[c, r, k, p, m] = xe[r, padded row c*512+m, k*128+p]
    xs = np.zeros((R, P4, KP), np.float16)
    xs[:, valid, :] = xe[:, rows[valid], :]
    xt = np.ascontiguousarray(
        xs.reshape(R, NCH, 4 * P, KT, P).transpose(1, 0, 3, 4, 2)
    )                                                         # [NCH, R, KT, P, 512]

    # wk[e, r, k, p, h] = w[r, e, k*128+p, h]; sliced per core along h
    wk = np.ascontiguousarray(w.transpose(1, 0, 2, 3)).reshape(E, R, KT, P, H)

    twp = np.zeros(NT * P, np.float32)
    twp[valid] = ent_tw[rows[valid]]
    twt = np.ascontiguousarray(twp.reshape(NT, P).T)          # [128, NT]

    dest = np.zeros(NT * P, np.int64)                         # pads -> row 0 (+0.0)
    dest[valid] = ent_tok[rows[valid]]
    sidx = np.ascontiguousarray(dest.reshape(NT, P).T).astype(np.int32)

    meta = dict(NT=NT, NCH=NCH, ept=ept)
    shared = dict(xt=xt, twt=twt, sidx=sidx)
    in_maps = []
    for c in range(NC):
        m = dict(shared)
        m["wk"] = np.ascontiguousarray(wk[..., c * HC:(c + 1) * HC])
        in_maps.append(m)
    return in_maps, meta


# ---------------------------------------------------------------- device program
def build_program(meta):
    NT, NCH, ept = meta["NT"], meta["NCH"], meta["ept"]

    nc = bass.Bass()
    xt = nc.dram_tensor("xt", [NCH, R, KT, P, 4 * P], FP16, kind="ExternalInput")
    wk = nc.dram_tensor("wk", [E, R, KT, P, HC], FP16, kind="ExternalInput")
    twt = nc.dram_tensor("twt", [P, NT], FP32, kind="ExternalInput")
    sidx = nc.dram_tensor("sidx", [P, NT], INT32, kind="ExternalInput")
    y = nc.dram_tensor("y", [T, HC], FP16, kind="ExternalOutput")

    with tile.TileContext(nc) as tc:
        with (
            tc.tile_pool(name="xpool", bufs=3) as xpool,
            tc.tile_pool(name="wpool", bufs=2) as wpool,
            tc.tile_pool(name="cpool", bufs=1) as cpool,
            tc.tile_pool(name="epool", bufs=6) as epool,
            tc.tile_pool(name="psum", bufs=8, space="PSUM") as psum,
        ):
            # Per-rank tiles; x rides the SP HWDGE ring, w the ACT ring,
            # so the prologue loads stream on both rings in parallel.
            def load_x_chunk(c):
                ts = {}
                for r in range(R):
                    t_ = xpool.tile([P, KT, 4 * P], FP16, tag=f"xt{r}")
                    nc.sync.dma_start(t_[:], xt[c, r])
                    ts[r] = t_
                return ts

            def load_w_expert(e):
                ts = {}
                for r in range(R):
                    t_ = wpool.tile([P, KT, HC], FP16, tag=f"wk{r}")
                    nc.scalar.dma_start(t_[:], wk[e, r])
                    ts[r] = t_
                return ts

            xt_sb = load_x_chunk(0)
            wk_sb = load_w_expert(int(ept[0]))
            cur_chunk, cur_e = 0, int(ept[0])

            twt_sb = cpool.tile([P, NT], FP32)
            nc.sync.dma_start(twt_sb[:], twt[:])
            sidx_sb = cpool.tile([P, NT], INT32)
            nc.sync.dma_start(sidx_sb[:], sidx[:])

            for t in range(NT):
                c, m0 = t // 4, (t % 4) * P
                if c != cur_chunk:
                    xt_sb = load_x_chunk(c)
                    cur_chunk = c
                e = int(ept[t])
                if e != cur_e:
                    wk_sb = load_w_expert(e)
                    cur_e = e
                ps = psum.tile([P, HC], FP32, tag="ps")
                i = 0
                for r in range(R):
                    for k in range(KT):
                        nc.tensor.matmul(
                            ps[:],
                            xt_sb[r][:, k, m0:m0 + P],
                            wk_sb[r][:, k, :],
                            start=(i == 0),
                            stop=(i == R * KT - 1),
                        )
                        i += 1
                ev = epool.tile([P, HC], FP16, tag="ev")
                nc.scalar.activation(
                    ev[:], ps[:], mybir.ActivationFunctionType.Copy,
                    scale=twt_sb[:, t:t + 1],
                )
                nc.gpsimd.indirect_dma_start(
                    out=y[:, :],
                    out_offset=IndirectOffsetOnAxis(ap=sidx_sb[:, t:t + 1], axis=0),
                    in_=ev[:], in_offset=None,
                    compute_op=mybir.AluOpType.add,
                )

    return nc


def assemble_output(results):
    return np.concatenate([results[c]["y"] for c in range(NC)], axis=1)


def kernel(intermediate_states, w, topk_ids, topk_weight):
    from concourse.bass_utils import run_bass_kernel_spmd

    install_drain_patch()
    in_maps, meta = prep_inputs(intermediate_states, w, topk_ids, topk_weight)
    nc = build_program(meta)
    split_multiwait(nc)
    res = run_bass_kernel_spmd(nc, in_maps, list(range(NC)))
    return assemble_output(res.results)


# revision 4
# speedup vs baseline: 1.1226x; 1.1226x over previous
"""MoE down-proj GEMM + reduce over tensor-parallel K-shards (the R axis),
as a Bass/Tile kernel SPMD across 8 TRN2 NeuronCores.

Problem (hardcoded from the spec):
    intermediate_states [4, 4096, 1024] f16, w [4, 8, 1024, 4096] f16,
    topk_ids [2048, 2] i32, topk_weight [2048, 2] f32 -> out [2048, 4096] f16

Strategy (no collectives needed):
- Tensor-parallel on the OUTPUT dim: every core owns a 512-wide slice of
  H; the reduce over the R axis (the module's tensor-parallel K-shards)
  happens inside each core's PSUM accumulation (4 ranks x 8 k-tiles = 32
  accumulating matmuls per 128-token m-tile).
- Host routing: the GEMM is linear in x, so a token whose TOPK=2 slots
  hit the SAME expert is pre-combined into one row tw1*x1+tw2*x2 (weight
  1.0). Entries are stably sorted by expert and each expert group padded
  to 128-row m-tiles, so each matmul's stationary operand is
  single-expert. Activations are pre-transposed to [k, m] lhsT layout.
- Eviction applies the router weight as a per-partition ACT scale (fp16)
  and indirect-scatter-ADDs the rows straight into the pre-zeroed output
  (DMA CCE add): the TOPK pair-sum happens inside the DMA. Within one
  scatter all target rows are distinct (after combining, a token has at
  most one slot per expert), and Tile serializes the scatter chain, so
  there are no read-modify-write races. Padded rows carry x=0 and
  therefore add exactly 0.0 wherever they land (row 0).
- No second pass; the host just concatenates the 8 H-slices.
"""
import numpy as np

import concourse.bass as bass
import concourse.mybir as mybir
from concourse import tile
from concourse.bass import IndirectOffsetOnAxis

R, T, TOPK, KP, E, H = 4, 2048, 2, 1024, 8, 4096
MT = T * TOPK
NC = 8
HC = H // NC             # 512 H-columns per core
KT = KP // 128           # 8 k-tiles
P = 128

FP16 = mybir.dt.float16
FP32 = mybir.dt.float32
INT32 = mybir.dt.int32


# ------------------------------------------------------------- walrus workarounds
def install_drain_patch():
    """TileContext's exit drain waits on the whole 27-proc global clock in
    one instruction; this walrus build rejects >1 sync-wait per instruction
    (and any ge-imm wait on a Drain). Split into single-wait SP nops."""
    import bass_rust

    def _split_drain_and_barrier(self, tick_clock, wait_clock):
        ScopedClock = bass_rust.ScopedClock
        g = list(tick_clock.global_clock)
        for i, v in enumerate(g):
            if v <= 0:
                continue
            vc = bass_rust.VectorClock([v if j == i else 0 for j in range(len(g))])
            nop = self.nc.sync.nop(nofuse=True, hint="drain_wait_split")
            wait_clock.add_sem_waits(nop.ins, ScopedClock({None: vc}))
        self.nc.sync.drain()
        self.nc.all_engine_barrier()
        assert self.sems is not None
        popped = self.nc._tile_sem_poison_stack.pop()
        assert popped is self._sem_poison
        self.nc.clear_and_free_semaphores(list(self.sems.allocated().values()))
        self.nc.all_engine_barrier()

    tile.TileContext._drain_and_barrier = _split_drain_and_barrier


def split_multiwait(nc):
    """Hoist extra sync-waits onto fresh same-engine nops inserted just
    before the instruction (walrus encodes at most one wait each)."""
    for bb in nc.main_func.blocks:
        insts = list(bb.instructions)
        if not any(
            ins.sync_info and ins.sync_info.on_wait and len(ins.sync_info.on_wait) > 1
            for ins in insts
        ):
            continue
        out = []
        for ins in insts:
            si = ins.sync_info
            waits = list(si.on_wait) if si and si.on_wait else []
            if len(waits) > 1:
                eng = nc.engines[ins.engine]
                for wx in waits[:-1]:
                    nop = eng.nop(nofuse=True, hint="wait_split").ins
                    cur = nc.cur_bb.bb
                    lst = cur.instructions
                    assert lst and lst[-1].name == nop.name
                    cur.instructions = lst[:-1]
                    nsi = nop.sync_info
                    if nsi is None:
                        nop.sync_info = mybir.SyncInfo(on_wait=[wx], on_update=[])
                    else:
                        nsi.on_wait = [wx]
                    out.append(nop)
                si.on_wait = waits[-1:]
            out.append(ins)
        bb.instructions = out


# ---------------------------------------------------------------- host routing
def prep_inputs(intermediate_states, w, topk_ids, topk_weight):
    x = np.asarray(intermediate_states)                       # [R, MT, KP] f16
    w = np.asarray(w)                                         # [R, E, KP, H] f16
    tw = np.asarray(topk_weight, np.float32).reshape(T, TOPK)
    ids = np.asarray(topk_ids, np.int64).reshape(T, TOPK)

    comb = ids[:, 0] == ids[:, 1]
    tcomb = np.nonzero(comb)[0]
    tsep = np.nonzero(~comb)[0]

    # entries: combined tokens (one row, weights folded into x), then the
    # two separate slots of mixed-expert tokens
    ent_e = np.concatenate([ids[tcomb, 0], ids[tsep, 0], ids[tsep, 1]])
    ent_tok = np.concatenate([tcomb, tsep, tsep])             # scatter target row
    n_ent = len(ent_e)

    xe = np.empty((R, n_ent, KP), np.float16)
    nco = len(tcomb)
    if nco:
        xe[:, :nco, :] = (
            x[:, 2 * tcomb, :].astype(np.float32) * tw[tcomb, 0][None, :, None]
            + x[:, 2 * tcomb + 1, :].astype(np.float32) * tw[tcomb, 1][None, :, None]
        ).astype(np.float16)
    xe[:, nco:nco + len(tsep), :] = x[:, 2 * tsep, :]
    xe[:, nco + len(tsep):, :] = x[:, 2 * tsep + 1, :]
    ent_tw = np.concatenate([np.ones(nco, np.float32), tw[tsep, 0], tw[tsep, 1]])

    # stable sort by expert; pad each expert group to 128-row m-tiles
    order = np.argsort(ent_e, kind="stable")
    ept = []                                                  # m-tile -> expert
    rows = []                                                 # padded row -> entry (-1 pad)
    for e in range(E):
        grp = order[ent_e[order] == e]
        n = len(grp)
        if n == 0:
            continue
        ntile = -(-n // P)
        padded = np.full(ntile * P, -1, np.int64)
        padded[:n] = grp
        rows.append(padded)
        ept.extend([e] * ntile)
    rows = np.concatenate(rows)
    NT = len(ept)
    NCH = -(-NT // 4)                                         # 512-row chunks
    P4 = NCH * 4 * P
    valid = np.nonzero(rows >= 0)[0]

    # xt[c, r, k, p, m] = xe[r, padded row c*512+m, k*128+p]
    xs = np.zeros((R, P4, KP), np.float16)
    xs[:, valid, :] = xe[:, rows[valid], :]
    xt = np.ascontiguousarray(
        xs.reshape(R, NCH, 4 * P, KT, P).transpose(1, 0, 3, 4, 2)
    )                                                         # [NCH, R, KT, P, 512]

    # wk[e, r, k, p, h] = w[r, e, k*128+p, h]; sliced per core along h
    wk = np.ascontiguousarray(w.transpose(1, 0, 2, 3)).reshape(E, R, KT, P, H)

    twp = np.zeros(NT * P, np.float32)
    twp[valid] = ent_tw[rows[valid]]
    twt = np.ascontiguousarray(twp.reshape(NT, P).T)          # [128, NT]

    dest = np.zeros(NT * P, np.int64)                         # pads -> row 0 (+0.0)
    dest[valid] = ent_tok[rows[valid]]
    sidx = np.ascontiguousarray(dest.reshape(NT, P).T).astype(np.int32)

    meta = dict(NT=NT, NCH=NCH, ept=ept)
    shared = dict(xt=xt, twt=twt, sidx=sidx)
    in_maps = []
    for c in range(NC):
        m = dict(shared)
        m["wk"] = np.ascontiguousarray(wk[..., c * HC:(c + 1) * HC])
        in_maps.append(m)
    return in_maps, meta


# ---------------------------------------------------------------- device program
def build_program(meta):
    NT, NCH, ept = meta["NT"], meta["NCH"], meta["ept"]

    nc = bass.Bass()
    xt = nc.dram_tensor("xt", [NCH, R, KT, P, 4 * P], FP16, kind="ExternalInput")
    wk = nc.dram_tensor("wk", [E, R, KT, P, HC], FP16, kind="ExternalInput")
    twt = nc.dram_tensor("twt", [P, NT], FP32, kind="ExternalInput")
    sidx = nc.dram_tensor("sidx", [P, NT], INT32, kind="ExternalInput")
    y = nc.dram_tensor("y", [T, HC], FP16, kind="ExternalOutput")

    with tile.TileContext(nc) as tc:
        with (
            tc.tile_pool(name="xpool", bufs=3) as xpool,
            tc.tile_pool(name="wpool", bufs=2) as wpool,
            tc.tile_pool(name="cpool", bufs=1) as cpool,
            tc.tile_pool(name="epool", bufs=6) as epool,
            tc.tile_pool(name="psum", bufs=7, space="PSUM") as psum,
        ):
            # Per-rank tiles; x rides the SP HWDGE ring, w the ACT ring,
            # so the prologue loads stream on both rings in parallel.
            def load_x_chunk(c):
                ts = {}
                for r in range(R):
                    t_ = xpool.tile([P, KT, 4 * P], FP16, tag=f"xt{r}")
                    nc.sync.dma_start(t_[:], xt[c, r])
                    ts[r] = t_
                return ts

            def load_w_expert(e):
                ts = {}
                for r in range(R):
                    t_ = wpool.tile([P, KT, HC], FP16, tag=f"wk{r}")
                    nc.scalar.dma_start(t_[:], wk[e, r])
                    ts[r] = t_
                return ts

            # PE warm-up treadmill: dep-free matmuls into a dead PSUM
            # bank keep the HAM clock-gate at 8/8 while the first 8MB of
            # inputs stream in, so real matmuls start at full clock.
            dummy = cpool.tile([P, 4 * P], FP16)
            nc.gpsimd.memset(dummy[:], 0.0)
            dps = psum.tile([P, HC], FP32, tag="warm", bufs=1)
            NWARM = 64
            for i in range(NWARM):
                nc.tensor.matmul(
                    dps[:], dummy[:, :P], dummy[:],
                    start=(i == 0), stop=(i == NWARM - 1),
                )

            xt_sb = load_x_chunk(0)
            wk_sb = load_w_expert(int(ept[0]))
            cur_chunk, cur_e = 0, int(ept[0])

            twt_sb = cpool.tile([P, NT], FP32)
            nc.sync.dma_start(twt_sb[:], twt[:])
            sidx_sb = cpool.tile([P, NT], INT32)
            nc.sync.dma_start(sidx_sb[:], sidx[:])

            for t in range(NT):
                c, m0 = t // 4, (t % 4) * P
                if c != cur_chunk:
                    xt_sb = load_x_chunk(c)
                    cur_chunk = c
                e = int(ept[t])
                if e != cur_e:
                    wk_sb = load_w_expert(e)
                    cur_e = e
                ps = psum.tile([P, HC], FP32, tag="ps")
                i = 0
                for r in range(R):
                    for k in range(KT):
                        nc.tensor.matmul(
                            ps[:],
                            xt_sb[r][:, k, m0:m0 + P],
                            wk_sb[r][:, k, :],
                            start=(i == 0),
                            stop=(i == R * KT - 1),
                        )
                        i += 1
                ev = epool.tile([P, HC], FP16, tag="ev")
                nc.scalar.activation(
                    ev[:], ps[:], mybir.ActivationFunctionType.Copy,
                    scale=twt_sb[:, t:t + 1],
                )
                nc.gpsimd.indirect_dma_start(
                    out=y[:, :],
                    out_offset=IndirectOffsetOnAxis(ap=sidx_sb[:, t:t + 1], axis=0),
                    in_=ev[:], in_offset=None,
                    compute_op=mybir.AluOpType.add,
                )

    return nc


def assemble_output(results):
    return np.concatenate([results[c]["y"] for c in range(NC)], axis=1)


def kernel(intermediate_states, w, topk_ids, topk_weight):
    from concourse.bass_utils import run_bass_kernel_spmd

    install_drain_patch()
    in_maps, meta = prep_inputs(intermediate_states, w, topk_ids, topk_weight)
    nc = build_program(meta)
    split_multiwait(nc)
    res = run_bass_kernel_spmd(nc, in_maps, list(range(NC)))
    return assemble_output(res.results)
